# revision 8
# baseline (speedup 1.0000x reference)
"""AdaptiveTrajectoryDecoder TRN2 Bass kernel (8 NeuronCores, pure data parallel).

Model (per sample, P=12 steps, H=256, E=64, D=2):
    emb   = relu(pos @ We.T + be)                     [E]
    gates = emb @ Wih.T + bih + h @ Whh.T + bhh       [4H]  (torch order i,f,g,o)
    c     = sig(f)*c + sig(i)*tanh(g);  h = sig(o)*tanh(c)
    pred  = pos + h @ Wpos.T + bpos;    pos = pred
    sp    = softplus(relu(h @ Wsp1.T + bsp1) @ Wsp2.T + bsp2)
    un    = exp(relu(h @ Wun1.T + bun1) @ Wun2.T + bun2)

Kernel layout: feature-major ([feature, batch] on chip). Batch 65536 is sharded
8 ways (8192/core), processed as 16 tiles of 512 columns. All matmuls run in
bf16 (f32 PSUM accumulate); c/h and the elementwise chain are bf16; pred/pos
and the sp/un pre-activations stay f32. sigmoid/tanh/relu run in the main loop
(one ACT table set); exp/softplus(=exp then ln(1+x)) run in a small tail pass
after a single table switch, on pre-activations staged through DRAM.

Gate weights are column-permuted host-side to (i, f, o, g) so each PSUM "duo"
([128, 1024] = 2 banks) holds one gate type and gets exactly one activation op.
b_ih+b_hh is folded into a constant-1 extra emb row (K=65 matmul); the other
biases ride the per-partition bias operand of relu / tensor_scalar ops.
"""

import numpy as np
import ml_dtypes

import concourse.bass as bass
import concourse.tile as tile
from concourse import mybir
from concourse.bass_utils import run_bass_kernel_spmd

BF = ml_dtypes.bfloat16
F32 = mybir.dt.float32
BF16 = mybir.dt.bfloat16
AF = mybir.ActivationFunctionType
ALU = mybir.AluOpType

N_CORES = 8
B, H, E, D, P = 65536, 256, 64, 2, 12
BC = B // N_CORES          # 8192 batch per core
NT = 512                   # batch tile (columns)
NJ = BC // NT              # 16 tiles per core
# permuted gate feature-block order: original blocks [i0 i1 f0 f1 g0 g1 o0 o1]
# -> [i0 i1 f0 f1 o0 o1 g0 g1] so duos are (i, f, o, g)
GATE_BLOCK_PERM = [0, 1, 2, 3, 6, 7, 4, 5]

_MAXW = 1  # max sem-waits this walrus build tolerates per instruction


def _patched_drain_and_barrier(self, tick_clock, wait_clock):
    """TileContext exit drain carries one wait per live semaphore; this walrus
    build rejects >2 waits on a Drain. Split them onto SP NoOps instead."""
    nc = self.nc
    probe = nc.sync.nop()
    wait_clock.add_sem_waits(probe.ins, tile.ScopedClock({None: tick_clock.global_clock}))
    si = probe.ins.sync_info
    waits = list(si.on_wait or []) if si else []
    probe.ins.sync_info = mybir.SyncInfo(on_wait=waits[:1], on_update=[])
    for i in range(1, len(waits)):
        extra = nc.sync.nop()
        extra.ins.sync_info = mybir.SyncInfo(on_wait=waits[i:i + 1], on_update=[])
    nc.sync.drain()
    nc.all_engine_barrier()
    assert self.sems is not None
    popped = nc._tile_sem_poison_stack.pop()
    assert popped is self._sem_poison
    nc.clear_and_free_semaphores(list(self.sems.allocated().values()))
    nc.all_engine_barrier()


tile.TileContext._drain_and_barrier = _patched_drain_and_barrier


def _split_excess_waits(nc, maxw=_MAXW):
    """Move excess sem-waits from any instruction onto same-engine NoOps
    inserted immediately before it (per-engine order preserved)."""
    n = 0
    for fn in nc.m.functions:
        for bb in fn.blocks:
            new_insts = []
            for inst in bb.instructions:
                si = getattr(inst, "sync_info", None)
                waits = list(si.on_wait) if si and si.on_wait else []
                if len(waits) > maxw:
                    chunks = [waits[i:i + maxw] for i in range(0, len(waits), maxw)]
                    for chunk in chunks[:-1]:
                        nop = mybir.InstNoOp(
                            name=f"waitsplit-{n}", ins=[], outs=[],
                            engine=inst.engine,
                            sync_info=mybir.SyncInfo(on_wait=chunk, on_update=[]),
                        )
                        n += 1
                        nc.register_instruction(nop, overwrite=True)
                        new_insts.append(nop)
                    inst.sync_info = mybir.SyncInfo(
                        on_wait=chunks[-1], on_update=list(si.on_update or []))
                new_insts.append(inst)
            bb.instructions[:] = new_insts


def build_nc():
    nc = bass.Bass()

    # ---- DRAM parameters (per-core shard shapes) ----
    h0 = nc.declare_dram_parameter("h0", [H, BC], BF16, isOutput=False)
    c0 = nc.declare_dram_parameter("c0", [H, BC], BF16, isOutput=False)
    pos0 = nc.declare_dram_parameter("pos0", [D, BC], F32, isOutput=False)
    wg = nc.declare_dram_parameter("wg", [H, 4 * H], BF16, isOutput=False)      # Whh.T col-perm
    wi65 = nc.declare_dram_parameter("wi65", [E + 1, 4 * H], BF16, isOutput=False)  # [Wih.T; bih+bhh] col-perm
    wspun1 = nc.declare_dram_parameter("wspun1", [H, 128], BF16, isOutput=False)    # [Wsp1;Wun1].T
    bspun1 = nc.declare_dram_parameter("bspun1", [128, 1], F32, isOutput=False)
    wpos5 = nc.declare_dram_parameter("wpos5", [H, 35], BF16, isOutput=False)   # cols 0:2 = Wpos.T
    wsu5 = nc.declare_dram_parameter("wsu5", [128, 35], BF16, isOutput=False)   # col 32 = Wsp2.T, 33:35 = Wun2.T
    b5 = nc.declare_dram_parameter("b5", [35, 1], F32, isOutput=False)          # rows 0:2 bpos, 32 bsp2, 33:35 bun2
    wemb = nc.declare_dram_parameter("wemb", [D, E], F32, isOutput=False)       # We.T
    bemb = nc.declare_dram_parameter("bemb", [E, 1], F32, isOutput=False)

    preds_out = nc.declare_dram_parameter("preds_out", [P, D, BC], F32, isOutput=True)
    speeds_out = nc.declare_dram_parameter("speeds_out", [P, BC], F32, isOutput=True)
    un_out = nc.declare_dram_parameter("un_out", [P, D, BC], F32, isOutput=True)

    with tile.TileContext(nc) as tc:
        with (
            tc.tile_pool(name="persist", bufs=1) as pp,
            tc.tile_pool(name="work", bufs=3) as wp,
            tc.tile_pool(name="ps", bufs=4, space="PSUM") as ps,
            tc.tile_pool(name="dstage", bufs=1, space="DRAM") as dp,
        ):
            # ---- persistent SBUF state ----
            h_sb = pp.tile([128, NJ * 2 * NT], BF16)    # per j: [h_half0 | h_half1]
            c_sb = pp.tile([128, NJ * 2 * NT], BF16)
            pos_sb = pp.tile([D, BC], F32)
            emb_sb = pp.tile([E + 1, NJ * NT], BF16)    # row E is constant 1.0

            wg_sb = pp.tile([128, 2, 4 * H], BF16)      # [k_half][gate feature col]
            wi_sb = pp.tile([E + 1, 4 * H], BF16)
            wspun1_sb = pp.tile([128, 2, 128], BF16)
            bspun1_sb = pp.tile([128, 1], F32)
            wpos5_sb = pp.tile([128, 2, 35], BF16)
            wsu5_sb = pp.tile([128, 35], BF16)
            b5_sb = pp.tile([35, 1], F32)
            wemb_sb = pp.tile([D, E], F32)
            bemb_sb = pp.tile([E, 1], F32)

            stage = dp.tile([P, 3, BC], F32)            # rows: sp_pre, un_pre0, un_pre1

            # ---- loads ----
            for k in range(2):
                nc.sync.dma_start(
                    h_sb.rearrange("p (j k n) -> p j k n", j=NJ, k=2)[:, :, k, :],
                    h0[k * 128:(k + 1) * 128, :].rearrange("p (j n) -> p j n", j=NJ))
                nc.sync.dma_start(
                    c_sb.rearrange("p (j k n) -> p j k n", j=NJ, k=2)[:, :, k, :],
                    c0[k * 128:(k + 1) * 128, :].rearrange("p (j n) -> p j n", j=NJ))
                nc.sync.dma_start(wg_sb[:, k, :], wg[k * 128:(k + 1) * 128, :])
                nc.sync.dma_start(wspun1_sb[:, k, :], wspun1[k * 128:(k + 1) * 128, :])
                nc.sync.dma_start(wpos5_sb[:, k, :], wpos5[k * 128:(k + 1) * 128, :])
            nc.sync.dma_start(pos_sb[:], pos0[:])
            nc.sync.dma_start(wi_sb[:], wi65[:])
            nc.sync.dma_start(bspun1_sb[:], bspun1[:])
            nc.sync.dma_start(wsu5_sb[:], wsu5[:])
            nc.sync.dma_start(b5_sb[:], b5[:])
            nc.sync.dma_start(wemb_sb[:], wemb[:])
            nc.sync.dma_start(bemb_sb[:], bemb[:])
            nc.vector.memset(emb_sb[E:E + 1, :], 1.0)

            def h_half(j, k):
                return h_sb[:, (2 * j + k) * NT:(2 * j + k + 1) * NT]

            def emb_j(j):
                return emb_sb[:, j * NT:(j + 1) * NT]

            def pos_j(j):
                return pos_sb[:, j * NT:(j + 1) * NT]

            def emit_emb(t, j):
                """emb(t, j) = relu(pred/pos (t-1, j) @ We.T + be); K=2 f32 matmul."""
                q = ps.tile([128, 1024], F32, tag="duo", name=f"embq_{t}_{j}")
                nc.tensor.matmul(q[0:E, 0:NT], wemb_sb[:], pos_j(j),
                                 start=True, stop=True)
                nc.vector.tensor_scalar(
                    out=emb_j(j)[0:E, :], in0=q[0:E, 0:NT],
                    scalar1=bemb_sb[:, 0:1], scalar2=0.0,
                    op0=ALU.add, op1=ALU.max)

            def emit_gates(t, j):
                """4 gate duos -> sig/tanh -> c,h update for tile j."""
                cj = c_sb[:, j * 2 * NT:(j + 1) * 2 * NT]
                hj = h_sb[:, j * 2 * NT:(j + 1) * 2 * NT]
                acts = []
                for d_idx, func in enumerate((AF.Sigmoid, AF.Sigmoid, AF.Sigmoid, AF.Tanh)):
                    q = ps.tile([128, 1024], F32, tag="duo", name=f"gq{d_idx}_{t}_{j}")
                    for half in range(2):
                        m = d_idx * 2 + half  # permuted feature block
                        o = q[:, half * NT:(half + 1) * NT]
                        nc.tensor.matmul(o, wg_sb[:, 0, m * 128:(m + 1) * 128],
                                         h_half(j, 0), start=True, stop=False)
                        nc.tensor.matmul(o, wg_sb[:, 1, m * 128:(m + 1) * 128],
                                         h_half(j, 1), start=False, stop=False)
                        nc.tensor.matmul(o, wi_sb[:, m * 128:(m + 1) * 128],
                                         emb_j(j), start=False, stop=True)
                    s = wp.tile([128, 1024], BF16, tag=f"act{d_idx}", name=f"s{d_idx}_{t}_{j}")
                    nc.scalar.activation(s[:], q[:], func)
                    acts.append(s)
                sig_i, sig_f, sig_o, tanh_g = acts
                t1 = wp.tile([128, 1024], BF16, tag="t1", name=f"t1_{t}_{j}")
                nc.vector.tensor_mul(t1[:], sig_f[:], cj)
                t2 = wp.tile([128, 1024], BF16, tag="t2", name=f"t2_{t}_{j}")
                nc.vector.tensor_mul(t2[:], sig_i[:], tanh_g[:])
                nc.vector.tensor_add(cj, t1[:], t2[:])
                tc_t = wp.tile([128, 1024], BF16, tag="tanh_c", name=f"tc_{t}_{j}")
                nc.scalar.activation(tc_t[:], cj, AF.Tanh)
                nc.vector.tensor_mul(hj, sig_o[:], tc_t[:])

            def emit_small(t, j):
                """spun1 + (pred|sp2|un2) + staging + next-step emb for tile j."""
                q = ps.tile([128, 1024], F32, tag="duo", name=f"smq_{t}_{j}")
                sp1 = q[:, 0:NT]
                nc.tensor.matmul(sp1, wspun1_sb[:, 0, :], h_half(j, 0),
                                 start=True, stop=False)
                nc.tensor.matmul(sp1, wspun1_sb[:, 1, :], h_half(j, 1),
                                 start=False, stop=True)
                spun = wp.tile([128, NT], BF16, tag="spun", name=f"spun_{t}_{j}")
                nc.vector.tensor_scalar(
                    out=spun[:], in0=sp1, scalar1=bspun1_sb[:, 0:1], scalar2=0.0,
                    op0=ALU.add, op1=ALU.max)
                p5 = q[0:35, NT:2 * NT]
                nc.tensor.matmul(p5, wpos5_sb[:, 0, :], h_half(j, 0),
                                 start=True, stop=False)
                nc.tensor.matmul(p5, wpos5_sb[:, 1, :], h_half(j, 1),
                                 start=False, stop=False)
                nc.tensor.matmul(p5, wsu5_sb[:], spun[:], start=False, stop=True)
                # pred = (p5[0:2] + bpos) + pos   (in-place into pos slice)
                nc.vector.scalar_tensor_tensor(
                    out=pos_j(j), in0=q[0:2, NT:2 * NT], scalar=b5_sb[0:2, 0:1],
                    in1=pos_j(j), op0=ALU.add, op1=ALU.add)
                # sp/un pre-activations -> SBUF -> DRAM staging
                st = wp.tile([3, NT], F32, tag="stage", name=f"st_{t}_{j}")
                nc.vector.tensor_scalar(
                    out=st[:], in0=q[32:35, NT:2 * NT], scalar1=b5_sb[32:35, 0:1],
                    scalar2=None, op0=ALU.add)
                nc.sync.dma_start(stage[t, :, j * NT:(j + 1) * NT], st[:])
                nc.sync.dma_start(preds_out[t, :, j * NT:(j + 1) * NT], pos_j(j))
                if t + 1 < P:
                    emit_emb(t + 1, j)

            # ---- prologue: emb(0, j) for all tiles ----
            for j in range(NJ):
                emit_emb(0, j)

            # ---- main loop, small-phase software-pipelined by one tile ----
            groups = [(t, j) for t in range(P) for j in range(NJ)]
            for n, (t, j) in enumerate(groups):
                emit_gates(t, j)
                if n >= 1:
                    emit_small(*groups[n - 1])
            emit_small(*groups[-1])

            # ---- tail: exp / softplus on staged pre-activations ----
            # sp: softplus(x) = ln(1 + exp(x));  un: exp(x)
            ch = BC // 128  # elems per partition per step (64 full config)
            spw = P * ch
            sp_in = wp.tile([128, spw], F32, tag="spin", bufs=1)
            for t in range(P):
                nc.sync.dma_start(sp_in[:, t * ch:(t + 1) * ch], stage[t, 0, :])
            sp_e = wp.tile([128, spw], F32, tag="spe", bufs=1)
            nc.scalar.activation(sp_e[:], sp_in[:], AF.Exp)
            sp_o = wp.tile([128, spw], F32, tag="spo", bufs=1)
            nc.scalar.activation(sp_o[:], sp_e[:], AF.Ln, bias=1.0)
            for t in range(P):
                nc.sync.dma_start(speeds_out[t, :], sp_o[:, t * ch:(t + 1) * ch])

            un_in = wp.tile([128, 2 * spw], F32, tag="unin", bufs=1)
            for t in range(P):
                nc.sync.dma_start(un_in[:, 2 * t * ch:2 * (t + 1) * ch],
                                  stage[t, 1:3, :])
            un_o = wp.tile([128, 2 * spw], F32, tag="uno", bufs=1)
            nc.scalar.activation(un_o[:], un_in[:], AF.Exp)
            for t in range(P):
                nc.sync.dma_start(un_out[t, :, :], un_o[:, 2 * t * ch:2 * (t + 1) * ch])

    _split_excess_waits(nc)
    return nc


_NC_CACHE = None


def _get_nc():
    global _NC_CACHE
    if _NC_CACHE is None:
        _NC_CACHE = build_nc()
    return _NC_CACHE


def kernel(encoder_hidden, encoder_cell, last_position,
           W_embed, b_embed, W_ih, W_hh, b_ih, b_hh,
           W_pos, b_pos, W_sp1, b_sp1, W_sp2, b_sp2,
           W_un1, b_un1, W_un2, b_un2):
    f32 = np.float32
    encoder_hidden = np.asarray(encoder_hidden, f32)
    encoder_cell = np.asarray(encoder_cell, f32)
    last_position = np.asarray(last_position, f32)

    perm = np.concatenate([np.arange(128) + 128 * b for b in GATE_BLOCK_PERM])

    wg_np = np.ascontiguousarray(np.asarray(W_hh, f32).T[:, perm]).astype(BF)
    bg = (np.asarray(b_ih, f32) + np.asarray(b_hh, f32))[perm]
    wi65_np = np.ascontiguousarray(
        np.concatenate([np.asarray(W_ih, f32).T[:, perm], bg[None, :]], axis=0)
    ).astype(BF)
    wspun1_np = np.ascontiguousarray(
        np.concatenate([np.asarray(W_sp1, f32), np.asarray(W_un1, f32)], 0).T
    ).astype(BF)
    bspun1_np = np.concatenate(
        [np.asarray(b_sp1, f32), np.asarray(b_un1, f32)])[:, None].astype(f32)
    wpos5_np = np.zeros((H, 35), f32)
    wpos5_np[:, 0:2] = np.asarray(W_pos, f32).T
    wpos5_np = wpos5_np.astype(BF)
    wsu5_np = np.zeros((128, 35), f32)
    wsu5_np[0:64, 32] = np.asarray(W_sp2, f32)[0]
    wsu5_np[64:128, 33:35] = np.asarray(W_un2, f32).T
    wsu5_np = wsu5_np.astype(BF)
    b5_np = np.zeros((35, 1), f32)
    b5_np[0:2, 0] = np.asarray(b_pos, f32)
    b5_np[32, 0] = np.asarray(b_sp2, f32)[0]
    b5_np[33:35, 0] = np.asarray(b_un2, f32)
    wemb_np = np.ascontiguousarray(np.asarray(W_embed, f32).T)
    bemb_np = np.asarray(b_embed, f32)[:, None]

    eh_t = np.ascontiguousarray(encoder_hidden.T).astype(BF)   # [H, B]
    ec_t = np.ascontiguousarray(encoder_cell.T).astype(BF)
    lp_t = np.ascontiguousarray(last_position.T)               # [D, B] f32

    in_maps = []
    for c in range(N_CORES):
        sl = slice(c * BC, (c + 1) * BC)
        in_maps.append({
            "h0": np.ascontiguousarray(eh_t[:, sl]),
            "c0": np.ascontiguousarray(ec_t[:, sl]),
            "pos0": np.ascontiguousarray(lp_t[:, sl]),
            "wg": wg_np, "wi65": wi65_np, "wspun1": wspun1_np,
            "bspun1": bspun1_np, "wpos5": wpos5_np, "wsu5": wsu5_np,
            "b5": b5_np, "wemb": wemb_np, "bemb": bemb_np,
        })

    nc = _get_nc()
    res = run_bass_kernel_spmd(nc, in_maps, core_ids=list(range(N_CORES)))

    preds = np.empty((B, P, D), f32)
    speeds = np.empty((B, P, 1), f32)
    uns = np.empty((B, P, D), f32)
    for c in range(N_CORES):
        sl = slice(c * BC, (c + 1) * BC)
        r = res.results[c]
        preds[sl] = np.transpose(r["preds_out"], (2, 0, 1))
        speeds[sl, :, 0] = r["speeds_out"].T
        uns[sl] = np.transpose(r["un_out"], (2, 0, 1))
    return preds, speeds, uns


# revision 9
# speedup vs baseline: 1.1983x; 1.1983x over previous
"""AdaptiveTrajectoryDecoder TRN2 Bass kernel (8 NeuronCores, pure data parallel).

Model (per sample, P=12 steps, H=256, E=64, D=2):
    emb   = relu(pos @ We.T + be)                     [E]
    gates = emb @ Wih.T + bih + h @ Whh.T + bhh       [4H]  (torch order i,f,g,o)
    c     = sig(f)*c + sig(i)*tanh(g);  h = sig(o)*tanh(c)
    pred  = pos + h @ Wpos.T + bpos;    pos = pred
    sp    = softplus(relu(h @ Wsp1.T + bsp1) @ Wsp2.T + bsp2)
    un    = exp(relu(h @ Wun1.T + bun1) @ Wun2.T + bun2)

Kernel layout: feature-major ([feature, batch] on chip). Batch 65536 is sharded
8 ways (8192/core), processed as 16 tiles of 512 columns. All matmuls run in
bf16 (f32 PSUM accumulate); c/h and the elementwise chain are bf16; pred/pos
and the sp/un pre-activations stay f32. sigmoid/tanh/relu run in the main loop
(one ACT table set); exp/softplus(=exp then ln(1+x)) run in a small tail pass
after a single table switch, on pre-activations staged through DRAM.

Gate weights are column-permuted host-side to (i, f, o, g) so each PSUM "duo"
([128, 1024] = 2 banks) holds one gate type and gets exactly one activation op.
b_ih+b_hh is folded into a constant-1 extra emb row (K=65 matmul); the other
biases ride the per-partition bias operand of relu / tensor_scalar ops.
"""

import numpy as np
import ml_dtypes

import concourse.bass as bass
import concourse.tile as tile
from concourse import mybir
from concourse.bass_utils import run_bass_kernel_spmd

BF = ml_dtypes.bfloat16
F32 = mybir.dt.float32
BF16 = mybir.dt.bfloat16
AF = mybir.ActivationFunctionType
ALU = mybir.AluOpType

N_CORES = 8
B, H, E, D, P = 65536, 256, 64, 2, 12
BC = B // N_CORES          # 8192 batch per core
NT = 512                   # batch tile (columns)
NJ = BC // NT              # 16 tiles per core
# permuted gate feature-block order: original blocks [i0 i1 f0 f1 g0 g1 o0 o1]
# -> [i0 i1 f0 f1 o0 o1 g0 g1] so duos are (i, f, o, g)
GATE_BLOCK_PERM = [0, 1, 2, 3, 6, 7, 4, 5]

_MAXW = 1  # max sem-waits this walrus build tolerates per instruction


def _patched_drain_and_barrier(self, tick_clock, wait_clock):
    """TileContext exit drain carries one wait per live semaphore; this walrus
    build rejects >2 waits on a Drain. Split them onto SP NoOps instead."""
    nc = self.nc
    probe = nc.sync.nop()
    wait_clock.add_sem_waits(probe.ins, tile.ScopedClock({None: tick_clock.global_clock}))
    si = probe.ins.sync_info
    waits = list(si.on_wait or []) if si else []
    probe.ins.sync_info = mybir.SyncInfo(on_wait=waits[:1], on_update=[])
    for i in range(1, len(waits)):
        extra = nc.sync.nop()
        extra.ins.sync_info = mybir.SyncInfo(on_wait=waits[i:i + 1], on_update=[])
    nc.sync.drain()
    nc.all_engine_barrier()
    assert self.sems is not None
    popped = nc._tile_sem_poison_stack.pop()
    assert popped is self._sem_poison
    nc.clear_and_free_semaphores(list(self.sems.allocated().values()))
    nc.all_engine_barrier()


tile.TileContext._drain_and_barrier = _patched_drain_and_barrier


def _split_excess_waits(nc, maxw=_MAXW):
    """Move excess sem-waits from any instruction onto same-engine NoOps
    inserted immediately before it (per-engine order preserved)."""
    n = 0
    for fn in nc.m.functions:
        for bb in fn.blocks:
            new_insts = []
            for inst in bb.instructions:
                si = getattr(inst, "sync_info", None)
                waits = list(si.on_wait) if si and si.on_wait else []
                if len(waits) > maxw:
                    chunks = [waits[i:i + maxw] for i in range(0, len(waits), maxw)]
                    for chunk in chunks[:-1]:
                        nop = mybir.InstNoOp(
                            name=f"waitsplit-{n}", ins=[], outs=[],
                            engine=inst.engine,
                            sync_info=mybir.SyncInfo(on_wait=chunk, on_update=[]),
                        )
                        n += 1
                        nc.register_instruction(nop, overwrite=True)
                        new_insts.append(nop)
                    inst.sync_info = mybir.SyncInfo(
                        on_wait=chunks[-1], on_update=list(si.on_update or []))
                new_insts.append(inst)
            bb.instructions[:] = new_insts


def build_nc():
    nc = bass.Bass()

    # ---- DRAM parameters (per-core shard shapes) ----
    h0 = nc.declare_dram_parameter("h0", [H, BC], BF16, isOutput=False)
    c0 = nc.declare_dram_parameter("c0", [H, BC], BF16, isOutput=False)
    pos0 = nc.declare_dram_parameter("pos0", [D, BC], F32, isOutput=False)
    wg = nc.declare_dram_parameter("wg", [H, 4 * H], BF16, isOutput=False)      # Whh.T col-perm
    wi65 = nc.declare_dram_parameter("wi65", [E + 1, 4 * H], BF16, isOutput=False)  # [Wih.T; bih+bhh] col-perm
    wspun1 = nc.declare_dram_parameter("wspun1", [H, 128], BF16, isOutput=False)    # [Wsp1;Wun1].T
    bspun1 = nc.declare_dram_parameter("bspun1", [128, 1], F32, isOutput=False)
    wpos5 = nc.declare_dram_parameter("wpos5", [H, 35], BF16, isOutput=False)   # cols 0:2 = Wpos.T
    wsu5 = nc.declare_dram_parameter("wsu5", [128, 35], BF16, isOutput=False)   # col 32 = Wsp2.T, 33:35 = Wun2.T
    b5 = nc.declare_dram_parameter("b5", [35, 1], F32, isOutput=False)          # rows 0:2 bpos, 32 bsp2, 33:35 bun2
    wemb = nc.declare_dram_parameter("wemb", [D, E], F32, isOutput=False)       # We.T
    bemb = nc.declare_dram_parameter("bemb", [E, 1], F32, isOutput=False)

    preds_out = nc.declare_dram_parameter("preds_out", [P, D, BC], F32, isOutput=True)
    speeds_out = nc.declare_dram_parameter("speeds_out", [P, BC], F32, isOutput=True)
    un_out = nc.declare_dram_parameter("un_out", [P, D, BC], F32, isOutput=True)

    with tile.TileContext(nc) as tc:
        with (
            tc.tile_pool(name="persist", bufs=1) as pp,
            tc.tile_pool(name="work", bufs=3) as wp,
            tc.tile_pool(name="ps", bufs=4, space="PSUM") as ps,
            tc.tile_pool(name="dstage", bufs=1, space="DRAM") as dp,
        ):
            # ---- persistent SBUF state ----
            h_sb = pp.tile([128, NJ * 2 * NT], BF16)    # per j: [h_half0 | h_half1]
            c_sb = pp.tile([128, NJ * 2 * NT], BF16)
            pos_sb = pp.tile([D, BC], F32)
            emb_sb = pp.tile([E + 1, NJ * NT], BF16)    # row E is constant 1.0

            wg_sb = pp.tile([128, 2, 4 * H], BF16)      # [k_half][gate feature col]
            wi_sb = pp.tile([E + 1, 4 * H], BF16)
            wspun1_sb = pp.tile([128, 2, 128], BF16)
            bspun1_sb = pp.tile([128, 1], F32)
            wpos5_sb = pp.tile([128, 2, 35], BF16)
            wsu5_sb = pp.tile([128, 35], BF16)
            b5_sb = pp.tile([35, 1], F32)
            wemb_sb = pp.tile([D, E], F32)
            bemb_sb = pp.tile([E, 1], F32)

            stage = dp.tile([P, 3, BC], F32)            # rows: sp_pre, un_pre0, un_pre1

            # ---- loads ----
            for k in range(2):
                nc.sync.dma_start(
                    h_sb.rearrange("p (j k n) -> p j k n", j=NJ, k=2)[:, :, k, :],
                    h0[k * 128:(k + 1) * 128, :].rearrange("p (j n) -> p j n", j=NJ))
                nc.sync.dma_start(
                    c_sb.rearrange("p (j k n) -> p j k n", j=NJ, k=2)[:, :, k, :],
                    c0[k * 128:(k + 1) * 128, :].rearrange("p (j n) -> p j n", j=NJ))
                nc.sync.dma_start(wg_sb[:, k, :], wg[k * 128:(k + 1) * 128, :])
                nc.sync.dma_start(wspun1_sb[:, k, :], wspun1[k * 128:(k + 1) * 128, :])
                nc.sync.dma_start(wpos5_sb[:, k, :], wpos5[k * 128:(k + 1) * 128, :])
            nc.sync.dma_start(pos_sb[:], pos0[:])
            nc.sync.dma_start(wi_sb[:], wi65[:])
            nc.sync.dma_start(bspun1_sb[:], bspun1[:])
            nc.sync.dma_start(wsu5_sb[:], wsu5[:])
            nc.sync.dma_start(b5_sb[:], b5[:])
            nc.sync.dma_start(wemb_sb[:], wemb[:])
            nc.sync.dma_start(bemb_sb[:], bemb[:])
            nc.vector.memset(emb_sb[E:E + 1, :], 1.0)

            def h_half(j, k):
                return h_sb[:, (2 * j + k) * NT:(2 * j + k + 1) * NT]

            def emb_j(j):
                return emb_sb[:, j * NT:(j + 1) * NT]

            def pos_j(j):
                return pos_sb[:, j * NT:(j + 1) * NT]

            def emit_emb(t, j):
                """emb(t, j) = relu(pred/pos (t-1, j) @ We.T + be); K=2 f32 matmul."""
                q = ps.tile([128, 1024], F32, tag="duo", name=f"embq_{t}_{j}")
                nc.tensor.matmul(q[0:E, 0:NT], wemb_sb[:], pos_j(j),
                                 start=True, stop=True)
                nc.vector.tensor_scalar(
                    out=emb_j(j)[0:E, :], in0=q[0:E, 0:NT],
                    scalar1=bemb_sb[:, 0:1], scalar2=0.0,
                    op0=ALU.add, op1=ALU.max)

            def emit_gates(t, j):
                """4 gate duos -> sig/tanh -> c,h update for tile j."""
                cj = c_sb[:, j * 2 * NT:(j + 1) * 2 * NT]
                hj = h_sb[:, j * 2 * NT:(j + 1) * 2 * NT]
                acts = []
                for d_idx, func in enumerate((AF.Sigmoid, AF.Sigmoid, AF.Sigmoid, AF.Tanh)):
                    q = ps.tile([128, 1024], F32, tag="duo", name=f"gq{d_idx}_{t}_{j}")
                    for half in range(2):
                        m = d_idx * 2 + half  # permuted feature block
                        o = q[:, half * NT:(half + 1) * NT]
                        nc.tensor.matmul(o, wg_sb[:, 0, m * 128:(m + 1) * 128],
                                         h_half(j, 0), start=True, stop=False)
                        nc.tensor.matmul(o, wg_sb[:, 1, m * 128:(m + 1) * 128],
                                         h_half(j, 1), start=False, stop=False)
                        nc.tensor.matmul(o, wi_sb[:, m * 128:(m + 1) * 128],
                                         emb_j(j), start=False, stop=True)
                    s = wp.tile([128, 1024], BF16, tag=f"act{d_idx}", name=f"s{d_idx}_{t}_{j}")
                    nc.scalar.activation(s[:], q[:], func)
                    acts.append(s)
                sig_i, sig_f, sig_o, tanh_g = acts
                t1 = wp.tile([128, 1024], BF16, tag="t1", name=f"t1_{t}_{j}")
                nc.vector.tensor_mul(t1[:], sig_f[:], cj)
                t2 = wp.tile([128, 1024], BF16, tag="t2", name=f"t2_{t}_{j}")
                nc.vector.tensor_mul(t2[:], sig_i[:], tanh_g[:])
                nc.vector.tensor_add(cj, t1[:], t2[:])
                return sig_o

            def emit_small(t, j, sig_o):
                """tanh_c/h + spun1 + (pred|sp2|un2) + staging + next emb, tile j.
                Runs one group after gates(t, j) so the ACT queue never blocks
                on the DVE c-chain (PSUM duos drain at ACT's own pace)."""
                cj = c_sb[:, j * 2 * NT:(j + 1) * 2 * NT]
                hj = h_sb[:, j * 2 * NT:(j + 1) * 2 * NT]
                tc_t = wp.tile([128, 1024], BF16, tag="tanh_c", name=f"tc_{t}_{j}")
                nc.scalar.activation(tc_t[:], cj, AF.Tanh)
                nc.vector.tensor_mul(hj, sig_o[:], tc_t[:])
                q = ps.tile([128, 1024], F32, tag="duo", name=f"smq_{t}_{j}")
                sp1 = q[:, 0:NT]
                nc.tensor.matmul(sp1, wspun1_sb[:, 0, :], h_half(j, 0),
                                 start=True, stop=False)
                nc.tensor.matmul(sp1, wspun1_sb[:, 1, :], h_half(j, 1),
                                 start=False, stop=True)
                spun = wp.tile([128, NT], BF16, tag="spun", name=f"spun_{t}_{j}")
                nc.vector.tensor_scalar(
                    out=spun[:], in0=sp1, scalar1=bspun1_sb[:, 0:1], scalar2=0.0,
                    op0=ALU.add, op1=ALU.max)
                p5 = q[0:35, NT:2 * NT]
                nc.tensor.matmul(p5, wpos5_sb[:, 0, :], h_half(j, 0),
                                 start=True, stop=False)
                nc.tensor.matmul(p5, wpos5_sb[:, 1, :], h_half(j, 1),
                                 start=False, stop=False)
                nc.tensor.matmul(p5, wsu5_sb[:], spun[:], start=False, stop=True)
                # pred = (p5[0:2] + bpos) + pos   (in-place into pos slice)
                nc.vector.scalar_tensor_tensor(
                    out=pos_j(j), in0=q[0:2, NT:2 * NT], scalar=b5_sb[0:2, 0:1],
                    in1=pos_j(j), op0=ALU.add, op1=ALU.add)
                # sp/un pre-activations -> SBUF -> DRAM staging
                st = wp.tile([3, NT], F32, tag="stage", name=f"st_{t}_{j}")
                nc.vector.tensor_scalar(
                    out=st[:], in0=q[32:35, NT:2 * NT], scalar1=b5_sb[32:35, 0:1],
                    scalar2=None, op0=ALU.add)
                nc.sync.dma_start(stage[t, :, j * NT:(j + 1) * NT], st[:])
                nc.sync.dma_start(preds_out[t, :, j * NT:(j + 1) * NT], pos_j(j))
                if t + 1 < P:
                    emit_emb(t + 1, j)

            # ---- prologue: emb(0, j) for all tiles ----
            for j in range(NJ):
                emit_emb(0, j)

            # ---- main loop, small-phase software-pipelined by one tile ----
            groups = [(t, j) for t in range(P) for j in range(NJ)]
            pend = None  # (t, j, sig_o) awaiting its small phase
            for n, (t, j) in enumerate(groups):
                so = emit_gates(t, j)
                if pend is not None:
                    emit_small(*pend)
                pend = (t, j, so)
            emit_small(*pend)

            # ---- tail: exp / softplus on staged pre-activations ----
            # sp: softplus(x) = ln(1 + exp(x));  un: exp(x)
            ch = BC // 128  # elems per partition per step (64 full config)
            spw = P * ch
            sp_in = wp.tile([128, spw], F32, tag="spin", bufs=1)
            for t in range(P):
                nc.sync.dma_start(sp_in[:, t * ch:(t + 1) * ch], stage[t, 0, :])
            sp_e = wp.tile([128, spw], F32, tag="spe", bufs=1)
            nc.scalar.activation(sp_e[:], sp_in[:], AF.Exp)
            sp_o = wp.tile([128, spw], F32, tag="spo", bufs=1)
            nc.scalar.activation(sp_o[:], sp_e[:], AF.Ln, bias=1.0)
            for t in range(P):
                nc.sync.dma_start(speeds_out[t, :], sp_o[:, t * ch:(t + 1) * ch])

            un_in = wp.tile([128, 2 * spw], F32, tag="unin", bufs=1)
            for t in range(P):
                nc.sync.dma_start(un_in[:, 2 * t * ch:2 * (t + 1) * ch],
                                  stage[t, 1:3, :])
            un_o = wp.tile([128, 2 * spw], F32, tag="uno", bufs=1)
            nc.scalar.activation(un_o[:], un_in[:], AF.Exp)
            for t in range(P):
                nc.sync.dma_start(un_out[t, :, :], un_o[:, 2 * t * ch:2 * (t + 1) * ch])

    _split_excess_waits(nc)
    return nc


_NC_CACHE = None


def _get_nc():
    global _NC_CACHE
    if _NC_CACHE is None:
        _NC_CACHE = build_nc()
    return _NC_CACHE


def kernel(encoder_hidden, encoder_cell, last_position,
           W_embed, b_embed, W_ih, W_hh, b_ih, b_hh,
           W_pos, b_pos, W_sp1, b_sp1, W_sp2, b_sp2,
           W_un1, b_un1, W_un2, b_un2):
    f32 = np.float32
    encoder_hidden = np.asarray(encoder_hidden, f32)
    encoder_cell = np.asarray(encoder_cell, f32)
    last_position = np.asarray(last_position, f32)

    perm = np.concatenate([np.arange(128) + 128 * b for b in GATE_BLOCK_PERM])

    wg_np = np.ascontiguousarray(np.asarray(W_hh, f32).T[:, perm]).astype(BF)
    bg = (np.asarray(b_ih, f32) + np.asarray(b_hh, f32))[perm]
    wi65_np = np.ascontiguousarray(
        np.concatenate([np.asarray(W_ih, f32).T[:, perm], bg[None, :]], axis=0)
    ).astype(BF)
    wspun1_np = np.ascontiguousarray(
        np.concatenate([np.asarray(W_sp1, f32), np.asarray(W_un1, f32)], 0).T
    ).astype(BF)
    bspun1_np = np.concatenate(
        [np.asarray(b_sp1, f32), np.asarray(b_un1, f32)])[:, None].astype(f32)
    wpos5_np = np.zeros((H, 35), f32)
    wpos5_np[:, 0:2] = np.asarray(W_pos, f32).T
    wpos5_np = wpos5_np.astype(BF)
    wsu5_np = np.zeros((128, 35), f32)
    wsu5_np[0:64, 32] = np.asarray(W_sp2, f32)[0]
    wsu5_np[64:128, 33:35] = np.asarray(W_un2, f32).T
    wsu5_np = wsu5_np.astype(BF)
    b5_np = np.zeros((35, 1), f32)
    b5_np[0:2, 0] = np.asarray(b_pos, f32)
    b5_np[32, 0] = np.asarray(b_sp2, f32)[0]
    b5_np[33:35, 0] = np.asarray(b_un2, f32)
    wemb_np = np.ascontiguousarray(np.asarray(W_embed, f32).T)
    bemb_np = np.asarray(b_embed, f32)[:, None]

    eh_t = np.ascontiguousarray(encoder_hidden.T).astype(BF)   # [H, B]
    ec_t = np.ascontiguousarray(encoder_cell.T).astype(BF)
    lp_t = np.ascontiguousarray(last_position.T)               # [D, B] f32

    in_maps = []
    for c in range(N_CORES):
        sl = slice(c * BC, (c + 1) * BC)
        in_maps.append({
            "h0": np.ascontiguousarray(eh_t[:, sl]),
            "c0": np.ascontiguousarray(ec_t[:, sl]),
            "pos0": np.ascontiguousarray(lp_t[:, sl]),
            "wg": wg_np, "wi65": wi65_np, "wspun1": wspun1_np,
            "bspun1": bspun1_np, "wpos5": wpos5_np, "wsu5": wsu5_np,
            "b5": b5_np, "wemb": wemb_np, "bemb": bemb_np,
        })

    nc = _get_nc()
    res = run_bass_kernel_spmd(nc, in_maps, core_ids=list(range(N_CORES)))

    preds = np.empty((B, P, D), f32)
    speeds = np.empty((B, P, 1), f32)
    uns = np.empty((B, P, D), f32)
    for c in range(N_CORES):
        sl = slice(c * BC, (c + 1) * BC)
        r = res.results[c]
        preds[sl] = np.transpose(r["preds_out"], (2, 0, 1))
        speeds[sl, :, 0] = r["speeds_out"].T
        uns[sl] = np.transpose(r["un_out"], (2, 0, 1))
    return preds, speeds, uns


# revision 12
# speedup vs baseline: 1.6889x; 1.4094x over previous
"""AdaptiveTrajectoryDecoder TRN2 Bass kernel (8 NeuronCores, pure data parallel).

Model (per sample, P=12 steps, H=256, E=64, D=2):
    emb   = relu(pos @ We.T + be)                     [E]
    gates = emb @ Wih.T + bih + h @ Whh.T + bhh       [4H]  (torch order i,f,g,o)
    c     = sig(f)*c + sig(i)*tanh(g);  h = sig(o)*tanh(c)
    pred  = pos + h @ Wpos.T + bpos;    pos = pred
    sp    = softplus(relu(h @ Wsp1.T + bsp1) @ Wsp2.T + bsp2)
    un    = exp(relu(h @ Wun1.T + bun1) @ Wun2.T + bun2)

Kernel layout: feature-major ([feature, batch] on chip). Batch 65536 is sharded
8 ways (8192/core), processed as 16 tiles of 512 columns. All matmuls run in
bf16 (f32 PSUM accumulate); c/h and the elementwise chain are bf16; pred/pos
and the sp/un pre-activations stay f32. sigmoid/tanh/relu run in the main loop
(one ACT table set); exp/softplus(=exp then ln(1+x)) run in a small tail pass
after a single table switch, on pre-activations staged through DRAM.

Gate weights are column-permuted host-side to (i, f, o, g) so each PSUM "duo"
([128, 1024] = 2 banks) holds one gate type and gets exactly one activation op.
b_ih+b_hh is folded into a constant-1 extra emb row (K=65 matmul); the other
biases ride the per-partition bias operand of relu / tensor_scalar ops.
"""

import numpy as np
import ml_dtypes

import concourse.bass as bass
import concourse.tile as tile
from concourse import mybir
from concourse.bass_utils import run_bass_kernel_spmd

BF = ml_dtypes.bfloat16
F32 = mybir.dt.float32
BF16 = mybir.dt.bfloat16
AF = mybir.ActivationFunctionType
ALU = mybir.AluOpType

N_CORES = 8
B, H, E, D, P = 65536, 256, 64, 2, 12
BC = B // N_CORES          # 8192 batch per core
NT = 512                   # batch tile (columns)
NJ = BC // NT              # 16 tiles per core
# permuted gate feature-block order: original blocks [i0 i1 f0 f1 g0 g1 o0 o1]
# -> [i0 i1 f0 f1 o0 o1 g0 g1] so duos are (i, f, o, g)
GATE_BLOCK_PERM = [0, 1, 2, 3, 6, 7, 4, 5]

_MAXW = 1  # max sem-waits this walrus build tolerates per instruction


def _patched_drain_and_barrier(self, tick_clock, wait_clock):
    """TileContext exit drain carries one wait per live semaphore; this walrus
    build rejects >2 waits on a Drain. Split them onto SP NoOps instead."""
    nc = self.nc
    probe = nc.sync.nop()
    wait_clock.add_sem_waits(probe.ins, tile.ScopedClock({None: tick_clock.global_clock}))
    si = probe.ins.sync_info
    waits = list(si.on_wait or []) if si else []
    probe.ins.sync_info = mybir.SyncInfo(on_wait=waits[:1], on_update=[])
    for i in range(1, len(waits)):
        extra = nc.sync.nop()
        extra.ins.sync_info = mybir.SyncInfo(on_wait=waits[i:i + 1], on_update=[])
    nc.sync.drain()
    nc.all_engine_barrier()
    assert self.sems is not None
    popped = nc._tile_sem_poison_stack.pop()
    assert popped is self._sem_poison
    nc.clear_and_free_semaphores(list(self.sems.allocated().values()))
    nc.all_engine_barrier()


tile.TileContext._drain_and_barrier = _patched_drain_and_barrier


def _split_excess_waits(nc, maxw=_MAXW):
    """Move excess sem-waits from any instruction onto same-engine NoOps
    inserted immediately before it (per-engine order preserved)."""
    n = 0
    for fn in nc.m.functions:
        for bb in fn.blocks:
            new_insts = []
            for inst in bb.instructions:
                si = getattr(inst, "sync_info", None)
                waits = list(si.on_wait) if si and si.on_wait else []
                if len(waits) > maxw:
                    chunks = [waits[i:i + maxw] for i in range(0, len(waits), maxw)]
                    for chunk in chunks[:-1]:
                        nop = mybir.InstNoOp(
                            name=f"waitsplit-{n}", ins=[], outs=[],
                            engine=inst.engine,
                            sync_info=mybir.SyncInfo(on_wait=chunk, on_update=[]),
                        )
                        n += 1
                        nc.register_instruction(nop, overwrite=True)
                        new_insts.append(nop)
                    inst.sync_info = mybir.SyncInfo(
                        on_wait=chunks[-1], on_update=list(si.on_update or []))
                new_insts.append(inst)
            bb.instructions[:] = new_insts


def build_nc():
    nc = bass.Bass()

    # ---- DRAM parameters (per-core shard shapes) ----
    h0 = nc.declare_dram_parameter("h0", [H, BC], BF16, isOutput=False)
    c0 = nc.declare_dram_parameter("c0", [H, BC], BF16, isOutput=False)
    pos0 = nc.declare_dram_parameter("pos0", [D, BC], F32, isOutput=False)
    wg = nc.declare_dram_parameter("wg", [H, 4 * H], BF16, isOutput=False)      # Whh.T col-perm
    wi65 = nc.declare_dram_parameter("wi65", [E + 1, 4 * H], BF16, isOutput=False)  # [Wih.T; bih+bhh] col-perm
    wspun1 = nc.declare_dram_parameter("wspun1", [H, 128], BF16, isOutput=False)    # [Wsp1;Wun1].T
    bspun1 = nc.declare_dram_parameter("bspun1", [128, 1], F32, isOutput=False)
    wpos5 = nc.declare_dram_parameter("wpos5", [H, 35], BF16, isOutput=False)   # cols 0:2 = Wpos.T
    wsu5 = nc.declare_dram_parameter("wsu5", [128, 35], BF16, isOutput=False)   # col 32 = Wsp2.T, 33:35 = Wun2.T
    b5 = nc.declare_dram_parameter("b5", [35, 1], F32, isOutput=False)          # rows 0:2 bpos, 32 bsp2, 33:35 bun2
    wemb = nc.declare_dram_parameter("wemb", [D, E], BF16, isOutput=False)      # We.T
    bemb = nc.declare_dram_parameter("bemb", [E, 1], F32, isOutput=False)

    preds_out = nc.declare_dram_parameter("preds_out", [P, D, BC], F32, isOutput=True)
    speeds_out = nc.declare_dram_parameter("speeds_out", [P, BC], F32, isOutput=True)
    un_out = nc.declare_dram_parameter("un_out", [P, D, BC], F32, isOutput=True)

    with tile.TileContext(nc) as tc:
        with (
            tc.tile_pool(name="persist", bufs=1) as pp,
            tc.tile_pool(name="work", bufs=3) as wp,
            tc.tile_pool(name="ps", bufs=4, space="PSUM") as ps,
            tc.tile_pool(name="dstage", bufs=1, space="DRAM") as dp,
        ):
            # ---- persistent SBUF state ----
            h_sb = pp.tile([128, NJ * 2 * NT], BF16)    # per j: [h_half0 | h_half1]
            c_sb = pp.tile([128, NJ * 2 * NT], BF16)
            pos_sb = pp.tile([D, BC], F32)
            emb_sb = pp.tile([E + 1, NJ * NT], BF16)    # row E is constant 1.0

            wg_sb = pp.tile([128, 2, 4 * H], BF16)      # [k_half][gate feature col]
            wi_sb = pp.tile([E + 1, 4 * H], BF16)
            wspun1_sb = pp.tile([128, 2, 128], BF16)
            bspun1_sb = pp.tile([128, 1], F32)
            wpos5_sb = pp.tile([128, 2, 35], BF16)
            wsu5_sb = pp.tile([128, 35], BF16)
            b5_sb = pp.tile([35, 1], F32)
            wemb_sb = pp.tile([D, E], BF16)
            bemb_sb = pp.tile([E, 1], F32)

            stage = dp.tile([P, 3, BC], F32)            # rows: sp_pre, un_pre0, un_pre1

            # ---- loads ----
            for k in range(2):
                nc.sync.dma_start(
                    h_sb.rearrange("p (j k n) -> p j k n", j=NJ, k=2)[:, :, k, :],
                    h0[k * 128:(k + 1) * 128, :].rearrange("p (j n) -> p j n", j=NJ))
                nc.sync.dma_start(
                    c_sb.rearrange("p (j k n) -> p j k n", j=NJ, k=2)[:, :, k, :],
                    c0[k * 128:(k + 1) * 128, :].rearrange("p (j n) -> p j n", j=NJ))
                nc.sync.dma_start(wg_sb[:, k, :], wg[k * 128:(k + 1) * 128, :])
                nc.sync.dma_start(wspun1_sb[:, k, :], wspun1[k * 128:(k + 1) * 128, :])
                nc.sync.dma_start(wpos5_sb[:, k, :], wpos5[k * 128:(k + 1) * 128, :])
            nc.sync.dma_start(pos_sb[:], pos0[:])
            nc.sync.dma_start(wi_sb[:], wi65[:])
            nc.sync.dma_start(bspun1_sb[:], bspun1[:])
            nc.sync.dma_start(wsu5_sb[:], wsu5[:])
            nc.sync.dma_start(b5_sb[:], b5[:])
            nc.sync.dma_start(wemb_sb[:], wemb[:])
            nc.sync.dma_start(bemb_sb[:], bemb[:])
            nc.vector.memset(emb_sb[E:E + 1, :], 1.0)

            def h_half(j, k):
                return h_sb[:, (2 * j + k) * NT:(2 * j + k + 1) * NT]

            def emb_j(j):
                return emb_sb[:, j * NT:(j + 1) * NT]

            def pos_j(j):
                return pos_sb[:, j * NT:(j + 1) * NT]

            def cast_pos(t, j):
                """bf16 copy of pos tile j for the next emb matmul."""
                pb = wp.tile([D, NT], BF16, tag="posbf", bufs=8, name=f"pb_{t}_{j}")
                nc.vector.tensor_copy(pb[:], pos_j(j))
                return pb

            def emit_emb(t, j, pos_bf):
                """emb(t, j) = relu(pos_bf @ We.T + be); K=2 bf16 matmul."""
                q = ps.tile([128, 1024], F32, tag="duo", name=f"embq_{t}_{j}")
                nc.tensor.matmul(q[0:E, 0:NT], wemb_sb[:], pos_bf[:],
                                 start=True, stop=True)
                nc.vector.tensor_scalar(
                    out=emb_j(j)[0:E, :], in0=q[0:E, 0:NT],
                    scalar1=bemb_sb[:, 0:1], scalar2=0.0,
                    op0=ALU.add, op1=ALU.max)

            def emit_gates(t, j):
                """4 gate duos -> sig/tanh -> c,h update for tile j."""
                cj = c_sb[:, j * 2 * NT:(j + 1) * 2 * NT]
                hj = h_sb[:, j * 2 * NT:(j + 1) * 2 * NT]
                acts = []
                for d_idx, func in enumerate((AF.Sigmoid, AF.Sigmoid, AF.Sigmoid, AF.Tanh)):
                    q = ps.tile([128, 1024], F32, tag="duo", name=f"gq{d_idx}_{t}_{j}")
                    for half in range(2):
                        m = d_idx * 2 + half  # permuted feature block
                        o = q[:, half * NT:(half + 1) * NT]
                        nc.tensor.matmul(o, wg_sb[:, 0, m * 128:(m + 1) * 128],
                                         h_half(j, 0), start=True, stop=False)
                        nc.tensor.matmul(o, wg_sb[:, 1, m * 128:(m + 1) * 128],
                                         h_half(j, 1), start=False, stop=False)
                        nc.tensor.matmul(o, wi_sb[:, m * 128:(m + 1) * 128],
                                         emb_j(j), start=False, stop=True)
                    s = wp.tile([128, 1024], BF16, tag=f"act{d_idx}", name=f"s{d_idx}_{t}_{j}")
                    nc.scalar.activation(s[:], q[:], func)
                    acts.append(s)
                sig_i, sig_f, sig_o, tanh_g = acts
                t1 = wp.tile([128, 1024], BF16, tag="t1", name=f"t1_{t}_{j}")
                nc.vector.tensor_mul(t1[:], sig_f[:], cj)
                t2 = wp.tile([128, 1024], BF16, tag="t2", name=f"t2_{t}_{j}")
                nc.vector.tensor_mul(t2[:], sig_i[:], tanh_g[:])
                nc.vector.tensor_add(cj, t1[:], t2[:])
                return sig_o

            def emit_small(t, j, sig_o):
                """tanh_c/h + spun1 + (pred|sp2|un2) + staging + next emb, tile j.
                Runs one group after gates(t, j) so the ACT queue never blocks
                on the DVE c-chain (PSUM duos drain at ACT's own pace)."""
                cj = c_sb[:, j * 2 * NT:(j + 1) * 2 * NT]
                hj = h_sb[:, j * 2 * NT:(j + 1) * 2 * NT]
                tc_t = wp.tile([128, 1024], BF16, tag="tanh_c", name=f"tc_{t}_{j}")
                nc.scalar.activation(tc_t[:], cj, AF.Tanh)
                nc.vector.tensor_mul(hj, sig_o[:], tc_t[:])
                q = ps.tile([128, 1024], F32, tag="duo", name=f"smq_{t}_{j}")
                sp1 = q[:, 0:NT]
                nc.tensor.matmul(sp1, wspun1_sb[:, 0, :], h_half(j, 0),
                                 start=True, stop=False)
                nc.tensor.matmul(sp1, wspun1_sb[:, 1, :], h_half(j, 1),
                                 start=False, stop=True)
                spun = wp.tile([128, NT], BF16, tag="spun", name=f"spun_{t}_{j}")
                nc.vector.tensor_scalar(
                    out=spun[:], in0=sp1, scalar1=bspun1_sb[:, 0:1], scalar2=0.0,
                    op0=ALU.add, op1=ALU.max)
                p5 = q[0:35, NT:2 * NT]
                nc.tensor.matmul(p5, wpos5_sb[:, 0, :], h_half(j, 0),
                                 start=True, stop=False)
                nc.tensor.matmul(p5, wpos5_sb[:, 1, :], h_half(j, 1),
                                 start=False, stop=False)
                nc.tensor.matmul(p5, wsu5_sb[:], spun[:], start=False, stop=True)
                # pred = (p5[0:2] + bpos) + pos   (in-place into pos slice)
                nc.vector.scalar_tensor_tensor(
                    out=pos_j(j), in0=q[0:2, NT:2 * NT], scalar=b5_sb[0:2, 0:1],
                    in1=pos_j(j), op0=ALU.add, op1=ALU.add)
                # sp/un pre-activations -> SBUF -> DRAM staging
                st = wp.tile([3, NT], F32, tag="stage", name=f"st_{t}_{j}")
                nc.vector.tensor_scalar(
                    out=st[:], in0=q[32:35, NT:2 * NT], scalar1=b5_sb[32:35, 0:1],
                    scalar2=None, op0=ALU.add)
                nc.sync.dma_start(stage[t, :, j * NT:(j + 1) * NT], st[:])
                nc.sync.dma_start(preds_out[t, :, j * NT:(j + 1) * NT], pos_j(j))
                if t + 1 < P:
                    return cast_pos(t, j)
                return None

            # ---- prologue: emb(0, j) for all tiles ----
            for j in range(NJ):
                emit_emb(0, j, cast_pos(-1, j))

            # ---- main loop, small-phase software-pipelined by one tile ----
            groups = [(t, j) for t in range(P) for j in range(NJ)]
            pend = None          # (t, j, sig_o) awaiting its small phase
            emb_q = []           # [(t+1, j, pos_bf)] delayed a few groups for PE
            emb_delay = min(2, max(0, NJ - 3))
            for n, (t, j) in enumerate(groups):
                while len(emb_q) > emb_delay:
                    emit_emb(*emb_q.pop(0))
                so = emit_gates(t, j)
                if pend is not None:
                    pt, pj, pso = pend
                    pb = emit_small(pt, pj, pso)
                    if pb is not None:
                        emb_q.append((pt + 1, pj, pb))
                pend = (t, j, so)
            emit_small(*pend)

            # ---- tail: exp / softplus on staged pre-activations ----
            # sp: softplus(x) = ln(1 + exp(x));  un: exp(x)
            ch = BC // 128  # elems per partition per step (64 full config)
            spw = P * ch
            sp_in = wp.tile([128, spw], F32, tag="spin", bufs=1)
            for t in range(P):
                nc.sync.dma_start(sp_in[:, t * ch:(t + 1) * ch], stage[t, 0, :])
            sp_e = wp.tile([128, spw], F32, tag="spe", bufs=1)
            nc.scalar.activation(sp_e[:], sp_in[:], AF.Exp)
            sp_o = wp.tile([128, spw], F32, tag="spo", bufs=1)
            nc.scalar.activation(sp_o[:], sp_e[:], AF.Ln, bias=1.0)
            for t in range(P):
                nc.sync.dma_start(speeds_out[t, :], sp_o[:, t * ch:(t + 1) * ch])

            un_in = wp.tile([128, 2 * spw], F32, tag="unin", bufs=1)
            for t in range(P):
                nc.sync.dma_start(un_in[:, 2 * t * ch:2 * (t + 1) * ch],
                                  stage[t, 1:3, :])
            un_o = wp.tile([128, 2 * spw], F32, tag="uno", bufs=1)
            nc.scalar.activation(un_o[:], un_in[:], AF.Exp)
            for t in range(P):
                nc.sync.dma_start(un_out[t, :, :], un_o[:, 2 * t * ch:2 * (t + 1) * ch])

    _split_excess_waits(nc)
    return nc


_NC_CACHE = None


def _get_nc():
    global _NC_CACHE
    if _NC_CACHE is None:
        _NC_CACHE = build_nc()
    return _NC_CACHE


def kernel(encoder_hidden, encoder_cell, last_position,
           W_embed, b_embed, W_ih, W_hh, b_ih, b_hh,
           W_pos, b_pos, W_sp1, b_sp1, W_sp2, b_sp2,
           W_un1, b_un1, W_un2, b_un2):
    f32 = np.float32
    encoder_hidden = np.asarray(encoder_hidden, f32)
    encoder_cell = np.asarray(encoder_cell, f32)
    last_position = np.asarray(last_position, f32)

    perm = np.concatenate([np.arange(128) + 128 * b for b in GATE_BLOCK_PERM])

    wg_np = np.ascontiguousarray(np.asarray(W_hh, f32).T[:, perm]).astype(BF)
    bg = (np.asarray(b_ih, f32) + np.asarray(b_hh, f32))[perm]
    wi65_np = np.ascontiguousarray(
        np.concatenate([np.asarray(W_ih, f32).T[:, perm], bg[None, :]], axis=0)
    ).astype(BF)
    wspun1_np = np.ascontiguousarray(
        np.concatenate([np.asarray(W_sp1, f32), np.asarray(W_un1, f32)], 0).T
    ).astype(BF)
    bspun1_np = np.concatenate(
        [np.asarray(b_sp1, f32), np.asarray(b_un1, f32)])[:, None].astype(f32)
    wpos5_np = np.zeros((H, 35), f32)
    wpos5_np[:, 0:2] = np.asarray(W_pos, f32).T
    wpos5_np = wpos5_np.astype(BF)
    wsu5_np = np.zeros((128, 35), f32)
    wsu5_np[0:64, 32] = np.asarray(W_sp2, f32)[0]
    wsu5_np[64:128, 33:35] = np.asarray(W_un2, f32).T
    wsu5_np = wsu5_np.astype(BF)
    b5_np = np.zeros((35, 1), f32)
    b5_np[0:2, 0] = np.asarray(b_pos, f32)
    b5_np[32, 0] = np.asarray(b_sp2, f32)[0]
    b5_np[33:35, 0] = np.asarray(b_un2, f32)
    wemb_np = np.ascontiguousarray(np.asarray(W_embed, f32).T).astype(BF)
    bemb_np = np.asarray(b_embed, f32)[:, None]

    eh_t = np.ascontiguousarray(encoder_hidden.T).astype(BF)   # [H, B]
    ec_t = np.ascontiguousarray(encoder_cell.T).astype(BF)
    lp_t = np.ascontiguousarray(last_position.T)               # [D, B] f32

    in_maps = []
    for c in range(N_CORES):
        sl = slice(c * BC, (c + 1) * BC)
        in_maps.append({
            "h0": np.ascontiguousarray(eh_t[:, sl]),
            "c0": np.ascontiguousarray(ec_t[:, sl]),
            "pos0": np.ascontiguousarray(lp_t[:, sl]),
            "wg": wg_np, "wi65": wi65_np, "wspun1": wspun1_np,
            "bspun1": bspun1_np, "wpos5": wpos5_np, "wsu5": wsu5_np,
            "b5": b5_np, "wemb": wemb_np, "bemb": bemb_np,
        })

    nc = _get_nc()
    res = run_bass_kernel_spmd(nc, in_maps, core_ids=list(range(N_CORES)))

    preds = np.empty((B, P, D), f32)
    speeds = np.empty((B, P, 1), f32)
    uns = np.empty((B, P, D), f32)
    for c in range(N_CORES):
        sl = slice(c * BC, (c + 1) * BC)
        r = res.results[c]
        preds[sl] = np.transpose(r["preds_out"], (2, 0, 1))
        speeds[sl, :, 0] = r["speeds_out"].T
        uns[sl] = np.transpose(r["un_out"], (2, 0, 1))
    return preds, speeds, uns
